# revision 13
# baseline (speedup 1.0000x reference)
"""Trainium2 Bass kernel for nn_CausalSE: causal cumulative-average pooling
+ squeeze-excite gating, data-parallel over batch (one NeuronCore per batch
element).

Reference math per batch element (D=512, T=8192, chunk=16, Tc=512):
    avg    = cumsum(x, t) / (t+1)
    pooled = avg[:, 15::16]                          # [D, Tc]
    h      = relu(w1 @ pooled + b1)                  # [64, Tc]
    g      = sigmoid(w2 @ h + b2)                    # [D, Tc]
    out    = repeat(g, 16, t)[:, :T] * x

The kernel is HBM-bound: per core it streams x in and out once.  x
crosses HBM as fp16 (host converts both ways), halving DMA to ~47us;
the SE bottleneck math stays fp32 (total error ~1e-3 of output scale vs
the 2e-2 gate).

Engine budget per core (measured rates drove every choice; DMA floor
~47us is the target for every engine's busy time):
  - DVE: 3-stage fp16 chunk-sum chain per d-tile (two strided halves
    adds at 380 G elem/s + j=4 windowed reduce) ~28us, scan + h-mul
    ~3us, and the d0/d1 gate multiplies (dense fp16 in-place
    tensor_tensor, 215 G elem/s) ~10us.
  - ACT: per-d-tile fused sigmoid+16x upsample reading the [128,CB]
    PSUM through a stride-0 broadcast view, writing the dense fp16 gate
    (~146 G elem/s, ~32us total) + relu.  No DMA on ACT.
  - GpSimd: d3 multiplies always, d2 for the big blocks (65 G elem/s),
    plus the d2/d3 stores via SWDGE.
  - SP ring: all loads up front, then the d0/d1 stores (FIFO behind the
    loads, which drain by ~30us).
"""

import sys

for _p in ("/opt/trn_rl_repo",):
    if _p not in sys.path:
        sys.path.insert(0, _p)

import numpy as np

B, D, T = 8, 512, 8192
DH = 64          # bottleneck dim = D // 8
CS = 16          # chunksize
TC = T // CS     # 512 chunks
NCORES = 8
NDT = D // 128   # 4 partition tiles of x / out
TBLOCKS = [(0, 1024), (1024, 2048), (3072, 2048), (5120, 2048),
           (7168, 512), (7680, 256), (7936, 256)]
TBMAX = 2048

_compiled_nc = None


def build_nc():
    import concourse.tile as tile
    from concourse import bacc, mybir

    f32 = mybir.dt.float32
    f16 = mybir.dt.float16
    AF = mybir.ActivationFunctionType
    ALU = mybir.AluOpType
    AX = mybir.AxisListType

    # Bacc (not plain Bass): its finalize() runs the TRN2 sync-wait
    # legalization (move_matmul_waits_to_ldweights / event-semaphore
    # splitting) that walrus codegen requires.
    nc = bacc.Bacc("TRN2", target_bir_lowering=False)
    x_d = nc.declare_dram_parameter("x", [D, T], f16, isOutput=False)
    w1t_d = nc.declare_dram_parameter("w1t", [D, DH], f16, isOutput=False)
    b1_d = nc.declare_dram_parameter("b1", [DH], f32, isOutput=False)
    w2t_d = nc.declare_dram_parameter("w2t", [DH, D], f32, isOutput=False)
    b2_d = nc.declare_dram_parameter("b2", [D], f32, isOutput=False)
    scale_d = nc.declare_dram_parameter("scale", [DH, TC], f32, isOutput=False)
    out_d = nc.declare_dram_parameter("out", [D, T], f16, isOutput=True)

    with tile.TileContext(nc) as tc:
        with (
            tc.tile_pool(name="xres", bufs=1) as xres,
            tc.tile_pool(name="small", bufs=1) as small,
            tc.tile_pool(name="ups", bufs=2) as ups,
            tc.tile_pool(name="red", bufs=2) as red,
            tc.tile_pool(name="psum_q", bufs=4, space="PSUM") as psum_q,
            tc.tile_pool(name="psum_g", bufs=4, space="PSUM") as psum_g,
        ):
            # x resident in SBUF: 4 tiles of [128, 8192] fp16 = 8 MB
            xt = [
                xres.tile([128, T], f16, tag=f"x{di}", name=f"x{di}")
                for di in range(NDT)
            ]
            st = [
                small.tile([128, TC], f16, tag=f"s{di}", name=f"s{di}")
                for di in range(NDT)
            ]
            w1s = small.tile([128, NDT, DH], f16, tag="w1")
            w2s = small.tile([DH, D], f32, tag="w2")
            b1s = small.tile([DH, 1], f32, tag="b1")
            b2s = small.tile([128, NDT], f32, tag="b2")
            scl = small.tile([DH, TC], f32, tag="scl")
            qs = small.tile([DH, TC], f32, tag="qs")    # causal prefix
            h = small.tile([DH, TC], f32, tag="h")

            # -- replicated weights / constants on the ACT queue (ACT is
            # idle at startup; keeps the two x-prefetch queues clean) --
            for ki in range(NDT):
                nc.scalar.dma_start(
                    w1s[:, ki, :], w1t_d[ki * 128:(ki + 1) * 128, :]
                )
                nc.scalar.dma_start(
                    b2s[:, ki:ki + 1],
                    b2_d[ki * 128:(ki + 1) * 128].unsqueeze(1),
                )
            nc.scalar.dma_start(w2s[:], w2t_d[:])
            nc.scalar.dma_start(b1s[:], b1_d[:].unsqueeze(1))
            nc.scalar.dma_start(scl[:], scale_d[:])

            # All loads issue up front, split over two rings (d0/d1 on the
            # SP HWDGE ring, d2/d3 via gpsimd SWDGE) so the 8 MB prefetch
            # pulls from HBM on two queues concurrently.  Stores are issued
            # on the same queues later and sit behind the loads in ring
            # order, which is fine: the loads drain early.
            for t0, TB in TBLOCKS:
                for di in range(NDT):
                    leng = nc.sync if di < 2 else nc.gpsimd
                    leng.dma_start(
                        xt[di][:, t0:t0 + TB],
                        x_d[di * 128:(di + 1) * 128, t0:t0 + TB],
                    )

            # Causal pipeline: gate for chunk c needs only x[:, :16(c+1)].
            # The gate multiplies for block k are emitted AFTER block k+1's
            # chunk-sum chain so the in-order DVE queue never stalls waiting
            # for the ACT sigmoid-upsample (software pipelining).
            deferred = None

            def emit_mults(items):
                for di, t0_, TB_, u_ in items:
                    xv = xt[di][:, t0_:t0_ + TB_]
                    nc.vector.tensor_tensor(xv, xv, u_[:, :TB_], op=ALU.mult)
                    # stores: d0/d1 on the SP ring (behind the loads),
                    # d2/d3 via gpsimd SWDGE.  GpSimd does NO compute: its
                    # SBUF port is shared with DVE and any Pool tensor op
                    # measurably poisons DVE throughput.
                    deng = nc.sync if di < 2 else nc.gpsimd
                    deng.dma_start(
                        out_d[di * 128:(di + 1) * 128, t0_:t0_ + TB_], xv
                    )

            for tb, (t0, TB) in enumerate(TBLOCKS):
                CB = TB // CS
                c0 = t0 // CS
                for di in range(NDT):
                    # chunk sums: for big blocks a 3-stage chain (halves +
                    # quarters TT adds at the 458 G read rate, then a j=4
                    # windowed reduce); for small blocks one windowed
                    # reduce (per-instruction overhead beats the chain)
                    v = xt[di][:, t0:t0 + TB].rearrange(
                        "p (c j) -> p c j", j=CS
                    )
                    if TB < 1024:
                        with nc.allow_low_precision(reason="f16 chunk sums"):
                            nc.vector.reduce_sum(
                                st[di][:, c0:c0 + CB], v, axis=AX.X
                            )
                        continue
                    r8 = red.tile(
                        [128, TBMAX // 2], f16, tag=f"r8_{di}", name=f"r8{di}"
                    )
                    v8 = r8[:, :CB * 8].rearrange("p (c j) -> p c j", j=8)
                    nc.vector.tensor_tensor(
                        v8, v[:, :, 0:8], v[:, :, 8:16], op=ALU.add
                    )
                    r4 = red.tile(
                        [128, TBMAX // 4], f16, tag=f"r4_{di}", name=f"r4{di}"
                    )
                    v4 = r4[:, :CB * 4].rearrange("p (c j) -> p c j", j=4)
                    nc.vector.tensor_tensor(
                        v4, v8[:, :, 0:4], v8[:, :, 4:8], op=ALU.add
                    )
                    with nc.allow_low_precision(reason="fp16 chunk sums"):
                        nc.vector.reduce_sum(
                            st[di][:, c0:c0 + CB], v4, axis=AX.X
                        )
                # q = w1 @ s for this block's chunk columns
                qp = psum_q.tile([DH, CB], f32, tag="q", name="qp")
                for ki in range(NDT):
                    nc.tensor.matmul(
                        qp[:],
                        w1s[:, ki, :],
                        st[ki][:, c0:c0 + CB],
                        start=(ki == 0),
                        stop=(ki == NDT - 1),
                    )
                # running causal prefix over this block (carry = last col)
                nc.vector.tensor_tensor_scan(
                    qs[:, c0:c0 + CB],
                    qp[:],
                    scl[:, c0:c0 + CB],
                    0.0 if tb == 0 else qs[:, c0 - 1:c0],
                    op0=ALU.add,
                    op1=ALU.bypass,
                )
                # SE bottleneck for this block's gate columns
                nc.vector.tensor_mul(
                    h[:, c0:c0 + CB], qs[:, c0:c0 + CB], scl[:, c0:c0 + CB]
                )
                nc.scalar.activation(
                    h[:, c0:c0 + CB], h[:, c0:c0 + CB], AF.Relu,
                    bias=b1s[:, :1],
                )
                last = tb == len(TBLOCKS) - 1
                if last and deferred is not None:
                    # flush the previous block's multiplies first so the
                    # tail drains in order
                    emit_mults(deferred)
                    deferred = None
                cur = []
                for di in range(NDT):
                    gp = psum_g.tile([128, CB], f32, tag="g", name="gp")
                    nc.tensor.matmul(
                        gp[:],
                        w2s[:, di * 128:(di + 1) * 128],
                        h[:, c0:c0 + CB],
                        start=True,
                        stop=True,
                    )
                    # fused sigmoid + 16x upsample: broadcast-read the
                    # PSUM column per chunk, write the dense fp16 gate
                    u = ups.tile(
                        [128, TBMAX], f16, tag=f"u{di}", name=f"u{di}"
                    )
                    nc.scalar.activation(
                        u[:, :TB].rearrange("p (c j) -> p c j", j=CS),
                        gp[:].unsqueeze(2).broadcast_to([128, CB, CS]),
                        AF.Sigmoid,
                        bias=b2s[:, di:di + 1],
                    )
                    if last:
                        # tail block: multiply right behind each sigmoid so
                        # the drain pipelines at d-tile granularity
                        emit_mults([(di, t0, TB, u)])
                    else:
                        cur.append((di, t0, TB, u))
                if deferred is not None:
                    emit_mults(deferred)
                deferred = cur if not last else None
    # run_bass_via_pjrt serializes nc.m as-is; Bacc defers register
    # allocation and TRN2 sync-wait legalization to finalize(), so it must
    # run here or walrus rejects the BIR.
    nc.finalize()
    return nc


def _host_inputs(x, w1, b1, w2, b2, chunksize):
    x = np.asarray(x)
    w1 = np.asarray(w1, dtype=np.float32)
    b1 = np.ascontiguousarray(np.asarray(b1, dtype=np.float32))
    w2 = np.asarray(w2, dtype=np.float32)
    b2 = np.ascontiguousarray(np.asarray(b2, dtype=np.float32))
    cs = int(chunksize)
    assert cs == CS and x.shape == (B, D, T), (cs, x.shape)
    x16 = np.ascontiguousarray(x.astype(np.float16))
    w1t = np.ascontiguousarray(w1.T.astype(np.float16))      # [D, DH]
    w2t = np.ascontiguousarray(w2.T)                         # [DH, D]
    scale = np.broadcast_to(
        1.0 / (CS * np.arange(1, TC + 1, dtype=np.float32)), (DH, TC)
    )
    scale = np.ascontiguousarray(scale)
    shared = dict(w1t=w1t, b1=b1, w2t=w2t, b2=b2, scale=scale)
    return x16, shared


def kernel(x, w1, b1, w2, b2, chunksize):
    global _compiled_nc
    from concourse.bass_utils import run_bass_kernel_spmd

    x16, shared = _host_inputs(x, w1, b1, w2, b2, chunksize)
    if _compiled_nc is None:
        _compiled_nc = build_nc()
    in_maps = [
        {"x": np.ascontiguousarray(x16[i]), **shared} for i in range(NCORES)
    ]
    res = run_bass_kernel_spmd(_compiled_nc, in_maps, list(range(NCORES)))
    out = np.stack(
        [res.results[i]["out"] for i in range(NCORES)], axis=0
    ).astype(np.float32)
    return out


# revision 14
# speedup vs baseline: 1.1460x; 1.1460x over previous
"""Trainium2 Bass kernel for nn_CausalSE: causal cumulative-average pooling
+ squeeze-excite gating, data-parallel over batch (one NeuronCore per batch
element).

Reference math per batch element (D=512, T=8192, chunk=16, Tc=512):
    avg    = cumsum(x, t) / (t+1)
    pooled = avg[:, 15::16]                          # [D, Tc]
    h      = relu(w1 @ pooled + b1)                  # [64, Tc]
    g      = sigmoid(w2 @ h + b2)                    # [D, Tc]
    out    = repeat(g, 16, t)[:, :T] * x

The kernel is HBM-bound: per core it streams x in and out once.  x
crosses HBM as fp16 (host converts both ways), halving DMA to ~47us;
the SE bottleneck math stays fp32 (total error ~1e-3 of output scale vs
the 2e-2 gate).

Engine budget per core (measured rates drove every choice; DMA floor
~47us is the target for every engine's busy time):
  - DVE: 3-stage fp16 chunk-sum chain per d-tile (two strided halves
    adds at 380 G elem/s + j=4 windowed reduce) ~28us, scan + h-mul
    ~3us, and the d0/d1 gate multiplies (dense fp16 in-place
    tensor_tensor, 215 G elem/s) ~10us.
  - ACT: per-d-tile fused sigmoid+16x upsample reading the [128,CB]
    PSUM through a stride-0 broadcast view, writing the dense fp16 gate
    (~146 G elem/s, ~32us total) + relu.  No DMA on ACT.
  - GpSimd: d3 multiplies always, d2 for the big blocks (65 G elem/s),
    plus the d2/d3 stores via SWDGE.
  - SP ring: all loads up front, then the d0/d1 stores (FIFO behind the
    loads, which drain by ~30us).
"""

import sys

for _p in ("/opt/trn_rl_repo",):
    if _p not in sys.path:
        sys.path.insert(0, _p)

import numpy as np

B, D, T = 8, 512, 8192
DH = 64          # bottleneck dim = D // 8
CS = 16          # chunksize
TC = T // CS     # 512 chunks
NCORES = 8
NDT = D // 128   # 4 partition tiles of x / out
TBLOCKS = [(0, 1024), (1024, 2048), (3072, 2048), (5120, 2048),
           (7168, 512), (7680, 256), (7936, 256)]
TBMAX = 2048

_compiled_nc = None


def build_nc():
    import concourse.tile as tile
    from concourse import bacc, mybir

    f32 = mybir.dt.float32
    f16 = mybir.dt.float16
    AF = mybir.ActivationFunctionType
    ALU = mybir.AluOpType
    AX = mybir.AxisListType

    # Bacc (not plain Bass): its finalize() runs the TRN2 sync-wait
    # legalization (move_matmul_waits_to_ldweights / event-semaphore
    # splitting) that walrus codegen requires.
    nc = bacc.Bacc("TRN2", target_bir_lowering=False)
    x_d = nc.declare_dram_parameter("x", [D, T], f16, isOutput=False)
    w1t_d = nc.declare_dram_parameter("w1t", [D, DH], f16, isOutput=False)
    b1_d = nc.declare_dram_parameter("b1", [DH], f32, isOutput=False)
    w2t_d = nc.declare_dram_parameter("w2t", [DH, D], f32, isOutput=False)
    b2_d = nc.declare_dram_parameter("b2", [D], f32, isOutput=False)
    scale_d = nc.declare_dram_parameter("scale", [DH, TC], f32, isOutput=False)
    out_d = nc.declare_dram_parameter("out", [D, T], f16, isOutput=True)

    with tile.TileContext(nc) as tc:
        with (
            tc.tile_pool(name="xres", bufs=1) as xres,
            tc.tile_pool(name="small", bufs=1) as small,
            tc.tile_pool(name="ups", bufs=2) as ups,
            tc.tile_pool(name="red", bufs=2) as red,
            tc.tile_pool(name="psum_q", bufs=4, space="PSUM") as psum_q,
            tc.tile_pool(name="psum_g", bufs=4, space="PSUM") as psum_g,
        ):
            # x resident in SBUF: 4 tiles of [128, 8192] fp16 = 8 MB
            xt = [
                xres.tile([128, T], f16, tag=f"x{di}", name=f"x{di}")
                for di in range(NDT)
            ]
            st = [
                small.tile([128, TC], f16, tag=f"s{di}", name=f"s{di}")
                for di in range(NDT)
            ]
            w1s = small.tile([128, NDT, DH], f16, tag="w1")
            w2s = small.tile([DH, D], f32, tag="w2")
            b1s = small.tile([DH, 1], f32, tag="b1")
            b2s = small.tile([128, NDT], f32, tag="b2")
            scl = small.tile([DH, TC], f32, tag="scl")
            qs = small.tile([DH, TC], f32, tag="qs")    # causal prefix
            h = small.tile([DH, TC], f32, tag="h")

            # -- replicated weights / constants on the ACT queue (ACT is
            # idle at startup; keeps the two x-prefetch queues clean) --
            for ki in range(NDT):
                nc.scalar.dma_start(
                    w1s[:, ki, :], w1t_d[ki * 128:(ki + 1) * 128, :]
                )
                nc.scalar.dma_start(
                    b2s[:, ki:ki + 1],
                    b2_d[ki * 128:(ki + 1) * 128].unsqueeze(1),
                )
            nc.scalar.dma_start(w2s[:], w2t_d[:])
            nc.scalar.dma_start(b1s[:], b1_d[:].unsqueeze(1))
            nc.scalar.dma_start(scl[:], scale_d[:])

            # All loads issue up front on the SP HWDGE ring (SWDGE descgen
            # is too slow to pace bulk loads).  The d0/d1 stores are issued
            # on the same queue later and sit behind the loads in ring
            # order, which is fine: the loads drain early.
            for t0, TB in TBLOCKS:
                for di in range(NDT):
                    nc.sync.dma_start(
                        xt[di][:, t0:t0 + TB],
                        x_d[di * 128:(di + 1) * 128, t0:t0 + TB],
                    )

            # Causal pipeline: gate for chunk c needs only x[:, :16(c+1)].
            # The gate multiplies for block k are emitted AFTER block k+1's
            # chunk-sum chain so the in-order DVE queue never stalls waiting
            # for the ACT sigmoid-upsample (software pipelining).
            deferred = None

            def emit_mults(items):
                for di, t0_, TB_, u_ in items:
                    xv = xt[di][:, t0_:t0_ + TB_]
                    nc.vector.tensor_tensor(xv, xv, u_[:, :TB_], op=ALU.mult)
                    # stores: d0/d1 on the SP ring (behind the loads),
                    # d2/d3 via gpsimd SWDGE.  GpSimd does NO compute: its
                    # SBUF port is shared with DVE and any Pool tensor op
                    # measurably poisons DVE throughput.
                    deng = nc.sync if di < 2 else nc.gpsimd
                    deng.dma_start(
                        out_d[di * 128:(di + 1) * 128, t0_:t0_ + TB_], xv
                    )

            for tb, (t0, TB) in enumerate(TBLOCKS):
                CB = TB // CS
                c0 = t0 // CS
                for di in range(NDT):
                    # chunk sums: for big blocks a 3-stage chain (halves +
                    # quarters TT adds at the 458 G read rate, then a j=4
                    # windowed reduce); for small blocks one windowed
                    # reduce (per-instruction overhead beats the chain)
                    v = xt[di][:, t0:t0 + TB].rearrange(
                        "p (c j) -> p c j", j=CS
                    )
                    if TB < 1024:
                        with nc.allow_low_precision(reason="f16 chunk sums"):
                            nc.vector.reduce_sum(
                                st[di][:, c0:c0 + CB], v, axis=AX.X
                            )
                        continue
                    r8 = red.tile(
                        [128, TBMAX // 2], f16, tag=f"r8_{di}", name=f"r8{di}"
                    )
                    v8 = r8[:, :CB * 8].rearrange("p (c j) -> p c j", j=8)
                    nc.vector.tensor_tensor(
                        v8, v[:, :, 0:8], v[:, :, 8:16], op=ALU.add
                    )
                    r4 = red.tile(
                        [128, TBMAX // 4], f16, tag=f"r4_{di}", name=f"r4{di}"
                    )
                    v4 = r4[:, :CB * 4].rearrange("p (c j) -> p c j", j=4)
                    nc.vector.tensor_tensor(
                        v4, v8[:, :, 0:4], v8[:, :, 4:8], op=ALU.add
                    )
                    with nc.allow_low_precision(reason="fp16 chunk sums"):
                        nc.vector.reduce_sum(
                            st[di][:, c0:c0 + CB], v4, axis=AX.X
                        )
                # q = w1 @ s for this block's chunk columns
                qp = psum_q.tile([DH, CB], f32, tag="q", name="qp")
                for ki in range(NDT):
                    nc.tensor.matmul(
                        qp[:],
                        w1s[:, ki, :],
                        st[ki][:, c0:c0 + CB],
                        start=(ki == 0),
                        stop=(ki == NDT - 1),
                    )
                # running causal prefix over this block (carry = last col)
                nc.vector.tensor_tensor_scan(
                    qs[:, c0:c0 + CB],
                    qp[:],
                    scl[:, c0:c0 + CB],
                    0.0 if tb == 0 else qs[:, c0 - 1:c0],
                    op0=ALU.add,
                    op1=ALU.bypass,
                )
                # SE bottleneck for this block's gate columns
                nc.vector.tensor_mul(
                    h[:, c0:c0 + CB], qs[:, c0:c0 + CB], scl[:, c0:c0 + CB]
                )
                nc.scalar.activation(
                    h[:, c0:c0 + CB], h[:, c0:c0 + CB], AF.Relu,
                    bias=b1s[:, :1],
                )
                last = tb == len(TBLOCKS) - 1
                if last and deferred is not None:
                    # flush the previous block's multiplies first so the
                    # tail drains in order
                    emit_mults(deferred)
                    deferred = None
                cur = []
                for di in range(NDT):
                    gp = psum_g.tile([128, CB], f32, tag="g", name="gp")
                    nc.tensor.matmul(
                        gp[:],
                        w2s[:, di * 128:(di + 1) * 128],
                        h[:, c0:c0 + CB],
                        start=True,
                        stop=True,
                    )
                    # fused sigmoid + 16x upsample: broadcast-read the
                    # PSUM column per chunk, write the dense fp16 gate
                    u = ups.tile(
                        [128, TBMAX], f16, tag=f"u{di}", name=f"u{di}"
                    )
                    nc.scalar.activation(
                        u[:, :TB].rearrange("p (c j) -> p c j", j=CS),
                        gp[:].unsqueeze(2).broadcast_to([128, CB, CS]),
                        AF.Sigmoid,
                        bias=b2s[:, di:di + 1],
                    )
                    if last:
                        # tail block: multiply right behind each sigmoid so
                        # the drain pipelines at d-tile granularity
                        emit_mults([(di, t0, TB, u)])
                    else:
                        cur.append((di, t0, TB, u))
                if deferred is not None:
                    emit_mults(deferred)
                deferred = cur if not last else None
    # run_bass_via_pjrt serializes nc.m as-is; Bacc defers register
    # allocation and TRN2 sync-wait legalization to finalize(), so it must
    # run here or walrus rejects the BIR.
    nc.finalize()
    return nc


def _host_inputs(x, w1, b1, w2, b2, chunksize):
    x = np.asarray(x)
    w1 = np.asarray(w1, dtype=np.float32)
    b1 = np.ascontiguousarray(np.asarray(b1, dtype=np.float32))
    w2 = np.asarray(w2, dtype=np.float32)
    b2 = np.ascontiguousarray(np.asarray(b2, dtype=np.float32))
    cs = int(chunksize)
    assert cs == CS and x.shape == (B, D, T), (cs, x.shape)
    x16 = np.ascontiguousarray(x.astype(np.float16))
    w1t = np.ascontiguousarray(w1.T.astype(np.float16))      # [D, DH]
    w2t = np.ascontiguousarray(w2.T)                         # [DH, D]
    scale = np.broadcast_to(
        1.0 / (CS * np.arange(1, TC + 1, dtype=np.float32)), (DH, TC)
    )
    scale = np.ascontiguousarray(scale)
    shared = dict(w1t=w1t, b1=b1, w2t=w2t, b2=b2, scale=scale)
    return x16, shared


def kernel(x, w1, b1, w2, b2, chunksize):
    global _compiled_nc
    from concourse.bass_utils import run_bass_kernel_spmd

    x16, shared = _host_inputs(x, w1, b1, w2, b2, chunksize)
    if _compiled_nc is None:
        _compiled_nc = build_nc()
    in_maps = [
        {"x": np.ascontiguousarray(x16[i]), **shared} for i in range(NCORES)
    ]
    res = run_bass_kernel_spmd(_compiled_nc, in_maps, list(range(NCORES)))
    out = np.stack(
        [res.results[i]["out"] for i in range(NCORES)], axis=0
    ).astype(np.float32)
    return out
